# revision 17
# baseline (speedup 1.0000x reference)
"""Trainium2 Bass kernel for the 2-layer heterogeneous GNN (GATv2 + CGConv).

Sharding: destination nodes (both node types) are split into 8 contiguous
ranges of 2560 (N padded 20000 -> 20480); each core owns the edges that
target its range, for all 4 relations.  Node features are replicated
(SBUF-resident, bf16, node-wrapped layout) so per-edge source gathers are
SBUF->SBUF dma_gather ops (feature-major output); destination-side
per-edge values come from one-hot selector matmuls on the PE.  The
one-hot matrices (static, from the edge lists) are precomputed on the
host and streamed from HBM.  The inter-layer halo exchange is a single
AllGather of the updated 2560-row slices.
"""

import os
import numpy as np
import ml_dtypes

BF = ml_dtypes.bfloat16

N = 20000
D = 128
H = 4
L = 2
E = 80000
CORES = 8
NPAD = 20480
SHARD = 2560
TILES = 20           # dst tiles of 128 per core
RANKS = NPAD // 128  # 160
PAD_NODE = 20000     # zero-feature padding node (valid gather target)
GTILES = 4           # dst tiles per gather chunk

LAST_EXEC_NS = None
DBG = None

# relation table: (name, kind, src_type, dst_type); cg before gat per dst type
RELS = [
    ("loses", "cg", "my", "opp"),
    ("beats", "gat", "my", "opp"),
    ("rev_beats", "cg", "opp", "my"),
    ("rev_loses", "gat", "opp", "my"),
]


# ----------------------------------------------------------------- host prep

def _wrap_nodes(x):
    """[N,128] f32 -> node-wrapped [128, RANKS*128] bf16 (node n at
    partition n%128, cols (n//128)*128 : +128)."""
    xp = np.zeros((NPAD, D), np.float32)
    xp[:N] = x
    return np.ascontiguousarray(
        xp.reshape(RANKS, 128, D).transpose(1, 0, 2).reshape(128, RANKS * D)
    ).astype(BF)


def _dst_major_slice(x, c):
    """core c's own dst slice, dst-major [128, TILES*128] bf16."""
    xp = np.zeros((NPAD, D), np.float32)
    xp[:N] = x
    sl = xp[c * SHARD:(c + 1) * SHARD]
    return np.ascontiguousarray(
        sl.reshape(TILES, 128, D).transpose(1, 0, 2).reshape(128, TILES * D)
    ).astype(BF)


def _feat_major_slice(x, c):
    """core c's own dst slice, feature-major [128, TILES*128] bf16
    (col t*128+j = node c*2560+t*128+j)."""
    xp = np.zeros((NPAD, D), np.float32)
    xp[:N] = x
    sl = xp[c * SHARD:(c + 1) * SHARD]  # [2560, D]
    return np.ascontiguousarray(
        sl.reshape(TILES, 128, D).transpose(2, 0, 1).reshape(D, TILES * 128)
    ).astype(BF)


def _prep_edges(ei):
    """bucket edges by (core, dst tile); returns per-core lists + max count."""
    src = np.asarray(ei[0]).astype(np.int64)
    dst = np.asarray(ei[1]).astype(np.int64)
    percore = []
    maxcnt = 1
    for c in range(CORES):
        m = (dst >= c * SHARD) & (dst < (c + 1) * SHARD)
        s, d = src[m], dst[m]
        dl = d - c * SHARD
        tid = dl // 128
        buckets = [np.nonzero(tid == t)[0] for t in range(TILES)]
        for b in buckets:
            maxcnt = max(maxcnt, len(b))
        percore.append((s, dl, buckets))
    return percore, maxcnt


def _pack_edges(percore, Bmax):
    """-> per-core (src_ids [EP], dloc [EP]); EP = TILES*Bmax*128, pad=-1."""
    out = []
    for (s, dl, buckets) in percore:
        src_a = np.full((TILES, Bmax * 128), PAD_NODE, np.int64)
        loc_a = np.full((TILES, Bmax * 128), -1, np.int64)
        for t, b in enumerate(buckets):
            n = len(b)
            src_a[t, :n] = s[b]
            loc_a[t, :n] = dl[b] % 128
        out.append((src_a.reshape(-1), loc_a.reshape(-1)))
    return out


def _onehots(loc, Bmax):
    """loc [EP] (-1 = pad) -> (oh_e [128, NB*128], oh_d [128, NB*128]) bf16.

    oh_e block gb: [j=edge-in-block, d=dst-local]; oh_d block = transpose."""
    NB = TILES * Bmax
    EP = NB * 128
    M = np.zeros((EP, 128), np.float32)
    valid = loc >= 0
    M[np.nonzero(valid)[0], loc[valid]] = 1.0
    Mb = M.reshape(NB, 128, 128)
    oh_e = np.ascontiguousarray(Mb.transpose(1, 0, 2).reshape(128, NB * 128))
    oh_d = np.ascontiguousarray(Mb.transpose(2, 0, 1).reshape(128, NB * 128))
    return oh_e.astype(BF), oh_d.astype(BF)


def _idx_dev(a):
    """[EP] int -> [128, EP//16] int16 (16-partition wrap, replicated 8x)."""
    x = a.astype(np.int16).reshape(-1, 16).T
    return np.ascontiguousarray(np.tile(x, (8, 1)))


def _rep(v, rows=128):
    return np.ascontiguousarray(
        np.tile(np.asarray(v, np.float32).reshape(1, -1), (rows, 1)))


# ------------------------------------------------------------- program build

def _build_program(Bmax):
    import concourse.bass as bass
    import concourse.bacc as bacc
    import concourse.mybir as mybir
    import concourse.tile as tile
    from concourse.hw_specs import get_activation_tables

    F32, BF16, I16 = mybir.dt.float32, mybir.dt.bfloat16, mybir.dt.int16
    AF = mybir.ActivationFunctionType
    OP = mybir.AluOpType

    NB = TILES * Bmax
    EP = NB * 128
    EPQ = GTILES * Bmax * 128       # idxs per gather chunk
    NCH = TILES // GTILES           # gather chunks per relation
    CB = GTILES * Bmax              # blocks per chunk

    k_layers = int(os.environ.get("K_LAYERS", str(L)))
    k_rels = os.environ.get("K_RELS", "")
    rels_active = [r for r in RELS if (not k_rels or r[0] in k_rels.split(","))]

    nc = bacc.Bacc("TRN2", target_bir_lowering=False, debug=False,
                   num_devices=CORES)

    dr = {}
    dr["xw_my"] = nc.dram_tensor("xw_my", [128, RANKS * D], BF16, kind="ExternalInput")
    dr["xw_opp"] = nc.dram_tensor("xw_opp", [128, RANKS * D], BF16, kind="ExternalInput")
    for ty in ("my", "opp"):
        dr[f"xres_{ty}"] = nc.dram_tensor(f"xres_{ty}", [128, TILES * D], BF16, kind="ExternalInput")
        dr[f"xfm_{ty}"] = nc.dram_tensor(f"xfm_{ty}", [128, TILES * 128], BF16, kind="ExternalInput")
    for rname, kind, _, _ in RELS:
        dr[f"si_{rname}"] = nc.dram_tensor(f"si_{rname}", [128, EP // 16], I16, kind="ExternalInput")
        dr[f"ohe_{rname}"] = nc.dram_tensor(f"ohe_{rname}", [128, NB * 128], BF16, kind="ExternalInput")
        dr[f"ohd_{rname}"] = nc.dram_tensor(f"ohd_{rname}", [128, NB * 128], BF16, kind="ExternalInput")
        if kind == "gat":
            dr[f"wl_{rname}"] = nc.dram_tensor(f"wl_{rname}", [L, 128, H * D], BF16, kind="ExternalInput")
            dr[f"wr_{rname}"] = nc.dram_tensor(f"wr_{rname}", [L, 128, H * D], BF16, kind="ExternalInput")
            dr[f"att_{rname}"] = nc.dram_tensor(f"att_{rname}", [L, 128, H * D], BF16, kind="ExternalInput")
            dr[f"gb_{rname}"] = nc.dram_tensor(f"gb_{rname}", [L, 128, D], F32, kind="ExternalInput")
        else:
            dr[f"wt_{rname}"] = nc.dram_tensor(f"wt_{rname}", [L, 128, 2 * D], BF16, kind="ExternalInput")
            dr[f"wb_{rname}"] = nc.dram_tensor(f"wb_{rname}", [L, 128, 2 * D], BF16, kind="ExternalInput")
            dr[f"cb_{rname}"] = nc.dram_tensor(f"cb_{rname}", [L, 1, 2 * D], BF16, kind="ExternalInput")
    dr["nw_w"] = nc.dram_tensor("nw_w", [L, 128, D], BF16, kind="ExternalInput")
    dr["nw_b"] = nc.dram_tensor("nw_b", [L, 128, 1], F32, kind="ExternalInput")
    dr["ident_f"] = nc.dram_tensor("ident_f", [128, 128], F32, kind="ExternalInput")
    dr["ident_b"] = nc.dram_tensor("ident_b", [128, 128], BF16, kind="ExternalInput")
    dr["out_my"] = nc.dram_tensor("out_my", [128, TILES * D], F32, kind="ExternalOutput")

    dr["out_opp"] = nc.dram_tensor("out_opp", [128, TILES * D], F32, kind="ExternalOutput")

    def ld3(pool, name, src, cols, dt=None):
        t = pool.tile([128, L * cols], dt or src.dtype, name=name, tag=name)
        nc.sync.dma_start(
            t[:].rearrange("p (l n) -> p l n", l=L),
            src[:].rearrange("l p n -> p l n"),
        )
        return t

    with tile.TileContext(nc) as tc:
        with tc.tile_pool(name="const", bufs=1) as cst, \
             tc.tile_pool(name="xwp", bufs=1) as xwp, \
             tc.tile_pool(name="accp", bufs=1) as accp, \
             tc.tile_pool(name="gth", bufs=3) as gth, \
             tc.tile_pool(name="ohp", bufs=2) as ohp, \
             tc.tile_pool(name="wrk", bufs=3) as wrk, \
             tc.tile_pool(name="til", bufs=2) as til, \
             tc.tile_pool(name="epi", bufs=1) as epi, \
             tc.tile_pool(name="dram", bufs=1, space="DRAM") as drm, \
             tc.tile_pool(name="pz", bufs=4, space=bass.MemorySpace.PSUM) as pzp, \
             tc.tile_pool(name="pagg", bufs=2, space=bass.MemorySpace.PSUM) as paggp:

            # one activation table serves Exp/Ln/Prelu/Copy/Identity
            tabs = list(get_activation_tables(nc.m.arch).items())
            need = {AF.Exp, AF.Ln, AF.Prelu, AF.Copy, AF.Identity}
            set_id = next(i for i, (_, fns) in enumerate(tabs) if need <= fns)
            nc.scalar.add_instruction(mybir.InstLoadActFuncSet(
                name=nc.get_next_instruction_name(), ins=[], outs=[],
                act_func_set_id=set_id))

            # ---------------- persistent SBUF state
            xw = {}
            for ty in ("my", "opp"):
                xw[ty] = xwp.tile([128, RANKS * D], BF16, name=f"xw_{ty}_sb", tag=f"xw_{ty}_sb")
                nc.sync.dma_start(xw[ty][:], dr[f"xw_{ty}"][:])
            xres, xfm = {}, {}
            for ty in ("my", "opp"):
                xres[ty] = xwp.tile([128, TILES * D], BF16, name=f"xres_{ty}_sb", tag=f"xres_{ty}_sb")
                nc.sync.dma_start(xres[ty][:], dr[f"xres_{ty}"][:])
                xfm[ty] = xwp.tile([128, TILES * 128], BF16, name=f"xfm_{ty}_sb", tag=f"xfm_{ty}_sb")
                nc.sync.dma_start(xfm[ty][:], dr[f"xfm_{ty}"][:])

            cw = {}
            for rname, kind, _, _ in RELS:
                si = cst.tile([128, EP // 16], I16, name=f"si_{rname}_sb", tag=f"si_{rname}_sb")
                nc.sync.dma_start(si[:], dr[f"si_{rname}"][:])
                cw[rname] = {"si": si}
                if kind == "gat":
                    cw[rname]["wl"] = ld3(cst, f"wl_{rname}_sb", dr[f"wl_{rname}"], H * D)
                    cw[rname]["wr"] = ld3(cst, f"wr_{rname}_sb", dr[f"wr_{rname}"], H * D)
                    cw[rname]["att"] = ld3(cst, f"att_{rname}_sb", dr[f"att_{rname}"], H * D)
                    cw[rname]["gb"] = ld3(cst, f"gb_{rname}_sb", dr[f"gb_{rname}"], D)
                else:
                    cw[rname]["wt"] = ld3(cst, f"wt_{rname}_sb", dr[f"wt_{rname}"], 2 * D)
                    cw[rname]["wb"] = ld3(cst, f"wb_{rname}_sb", dr[f"wb_{rname}"], 2 * D)
                    cbt = cst.tile([1, L * 2 * D], BF16, name=f"cb_{rname}_sb", tag=f"cb_{rname}_sb")
                    nc.sync.dma_start(
                        cbt[:].rearrange("p (l n) -> p l n", l=L),
                        dr[f"cb_{rname}"][:].rearrange("l p n -> p l n"),
                    )
                    cw[rname]["cb"] = cbt
            nw_w = ld3(cst, "nw_w_sb", dr["nw_w"], D)
            nw_b = ld3(cst, "nw_b_sb", dr["nw_b"], 1)
            ident_f = cst.tile([128, 128], F32, name="identf_sb", tag="identf_sb")
            nc.sync.dma_start(ident_f[:], dr["ident_f"][:])
            ident_b = cst.tile([128, 128], BF16, name="identb_sb", tag="identb_sb")
            nc.sync.dma_start(ident_b[:], dr["ident_b"][:])
            ones_b = cst.tile([1, 128], BF16, name="ones_sb", tag="ones_sb")
            nc.gpsimd.memset(ones_b[:], 1.0)

            # ---------------- layers
            def gather_chunk(rname, sty, l, q):
                cwr = cw[rname]
                xs = gth.tile([128, EPQ], BF16, name=f"xs_{rname}_{l}_{q}", tag="xs")
                nc.gpsimd.dma_gather(
                    out_ap=xs[:].rearrange("p (o n) -> p o n", o=1),
                    in_ap=xw[sty][:],
                    idxs_ap=cwr["si"][:, q * (EPQ // 16):(q + 1) * (EPQ // 16)],
                    num_idxs=EPQ, num_idxs_reg=EPQ,
                    elem_size=128, transpose=True,
                    single_packet=False,
                    sbuf_tokens_per_rank=128,
                    sbuf_free_dim_per_rank=256,
                    sbuf_free_dim_pad_per_rank=0,
                    sbuf_byte_offset=0,
                )
                ohe = ohp.tile([128, CB * 128], BF16, name=f"ohe_{rname}_{l}_{q}", tag="ohe")
                nc.sync.dma_start(ohe[:], dr[f"ohe_{rname}"][:, q * CB * 128:(q + 1) * CB * 128])
                ohd = ohp.tile([128, CB * 128], BF16, name=f"ohd_{rname}_{l}_{q}", tag="ohd")
                nc.scalar.dma_start(ohd[:], dr[f"ohd_{rname}"][:, q * CB * 128:(q + 1) * CB * 128])
                return xs, ohe, ohd

            def gat_tile(rname, dty, l, t, tq, ACC, xs, ohe_c, ohd_c):
                cwr = cw[rname]
                pxr = pzp.tile([128, H * D], F32, name=f"pxr_{rname}_{l}_{t}", tag="pz")
                nc.tensor.matmul(pxr[:], xfm[dty][:, t * 128:(t + 1) * 128],
                                 cwr["wr"][:, l * H * D:(l + 1) * H * D],
                                 start=True, stop=True)
                xr_sb = til.tile([128, H * D], BF16, name=f"xrsb_{rname}_{l}_{t}", tag="xr_sb")
                nc.scalar.copy(xr_sb[:], pxr[:])
                pagg = paggp.tile([128, H * D], F32, name=f"pagg_{rname}_{l}_{t}", tag="pagg")
                pden = paggp.tile([128, H], F32, name=f"pden_{rname}_{l}_{t}", tag="pden")

                pend = []
                for b in range(Bmax):
                    off = (tq * Bmax + b) * 128
                    xs_fm = xs[:, off:off + 128]
                    ohe = ohe_c[:, off:off + 128]
                    ohd = ohd_c[:, off:off + 128]
                    first, last = (b == 0), (b == Bmax - 1)
                    # psz_l: Wl part only (for alpha-weighted aggregation)
                    psz_l = pzp.tile([128, H * D], F32, name=f"pszl_{rname}_{l}_{t}_{b}", tag="pz")
                    nc.tensor.matmul(psz_l[:], xs_fm,
                                     cwr["wl"][:, l * H * D:(l + 1) * H * D],
                                     start=True, stop=True)
                    # psz_f: Wl + Wr[dst] (for the score)
                    psz_f = pzp.tile([128, H * D], F32, name=f"pszf_{rname}_{l}_{t}_{b}", tag="pz")
                    nc.tensor.matmul(psz_f[:], ohd, xr_sb[:],
                                     start=True, stop=False)
                    nc.tensor.matmul(psz_f[:], xs_fm,
                                     cwr["wl"][:, l * H * D:(l + 1) * H * D],
                                     start=False, stop=True)
                    z = wrk.tile([128, H * D], BF16, name=f"z_{rname}_{l}_{t}_{b}", tag="z")
                    nc.scalar.activation(z[:], psz_f[:], AF.Prelu, alpha=0.2)
                    scp = wrk.tile([128, H * D], BF16, name=f"scp_{rname}_{l}_{t}_{b}", tag="scp")
                    nc.vector.tensor_tensor(
                        scp[:], z[:],
                        cwr["att"][:, l * H * D:(l + 1) * H * D], op=OP.mult)
                    sc = wrk.tile([128, H], BF16, name=f"sc_{rname}_{l}_{t}_{b}", tag="sc")
                    with nc.allow_low_precision(reason="softmax logits tolerate bf16"):
                        nc.vector.tensor_reduce(
                            sc[:], scp[:].rearrange("p (h f) -> p h f", f=D),
                            axis=mybir.AxisListType.X, op=OP.add)
                    es = wrk.tile([128, H], BF16, name=f"es_{rname}_{l}_{t}_{b}", tag="es")
                    nc.scalar.activation(es[:], sc[:], AF.Exp)
                    # xlw = xl * es[h]  (broadcast along feature dim)
                    xlw = wrk.tile([128, H * D], BF16, name=f"xlw_{rname}_{l}_{t}_{b}", tag="xlw")
                    nc.vector.tensor_tensor(
                        xlw[:].rearrange("p (h f) -> p h f", f=D),
                        psz_l[:].rearrange("p (h f) -> p h f", f=D),
                        es[:].unsqueeze(2).broadcast_to((128, H, D)),
                        op=OP.mult)
                    if pend:
                        pend.pop(0)()
                    pend.append(
                        (lambda ohe=ohe, xlw=xlw, es=es, first=first, last=last: (
                            nc.tensor.matmul(pagg[:], ohe, xlw[:], start=first, stop=last),
                            nc.tensor.matmul(pden[:], ohe, es[:], start=first, stop=last))))
                while pend:
                    pend.pop(0)()

                # -------- tile epilogue (gat adds after cg wrote ACC)
                asl = ACC[dty][:, t * D:(t + 1) * D]
                sden = til.tile([128, H], F32, name=f"sden_{rname}_{l}_{t}", tag="sden")
                nc.vector.tensor_scalar(sden[:], pden[:], 1e-16, 4.0,
                                        op0=OP.add, op1=OP.mult)
                inv4 = til.tile([128, H], F32, name=f"inv4_{rname}_{l}_{t}", tag="inv4")
                nc.vector.reciprocal_approx_fast(inv4[:], sden[:])
                gtmp = til.tile([128, H * D], F32, name=f"gtmp_{rname}_{l}_{t}", tag="gtmp")
                nc.vector.tensor_tensor(
                    gtmp[:].rearrange("p (h f) -> p h f", f=D),
                    pagg[:].rearrange("p (h f) -> p h f", f=D),
                    inv4[:].unsqueeze(2).broadcast_to((128, H, D)),
                    op=OP.mult)
                gt = til.tile([128, D], F32, name=f"gt_{rname}_{l}_{t}", tag="gt")
                nc.vector.tensor_reduce(
                    gt[:], gtmp[:].rearrange("p (h f) -> p f h", f=D),
                    axis=mybir.AxisListType.X, op=OP.add)
                gt2 = til.tile([128, D], F32, name=f"gt2_{rname}_{l}_{t}", tag="gt2")
                nc.vector.scalar_tensor_tensor(
                    gt2[:], gt[:], 1.0, cwr["gb"][:, l * D:(l + 1) * D],
                    op0=OP.mult, op1=OP.add)
                nc.vector.tensor_tensor(asl, asl, gt2[:], op=OP.add)

            def cg_tile(rname, dty, l, t, tq, ACC, xs, ohe_c, ohd_c):
                cwr = cw[rname]
                pud = pzp.tile([128, 2 * D], F32, name=f"pud_{rname}_{l}_{t}", tag="pz")
                nc.tensor.matmul(pud[:], xfm[dty][:, t * 128:(t + 1) * 128],
                                 cwr["wt"][:, l * 2 * D:(l + 1) * 2 * D],
                                 start=True, stop=False)
                nc.tensor.matmul(pud[:], ones_b[:],
                                 cwr["cb"][:, l * 2 * D:(l + 1) * 2 * D],
                                 start=False, stop=True)
                ud_sb = til.tile([128, 2 * D], BF16, name=f"udsb_{rname}_{l}_{t}", tag="ud_sb")
                nc.scalar.copy(ud_sb[:], pud[:])
                pagg = paggp.tile([128, D], F32, name=f"pagg_{rname}_{l}_{t}", tag="pagg")

                pend = []
                for b in range(Bmax):
                    off = (tq * Bmax + b) * 128
                    xs_fm = xs[:, off:off + 128]
                    ohe = ohe_c[:, off:off + 128]
                    ohd = ohd_c[:, off:off + 128]
                    first, last = (b == 0), (b == Bmax - 1)
                    psm = pzp.tile([128, 2 * D], F32, name=f"psm_{rname}_{l}_{t}_{b}", tag="pz")
                    nc.tensor.matmul(psm[:], ohd, ud_sb[:],
                                     start=True, stop=False)
                    nc.tensor.matmul(psm[:], xs_fm,
                                     cwr["wb"][:, l * 2 * D:(l + 1) * 2 * D],
                                     start=False, stop=True)
                    # cols 0:D hold -u; cols D:2D hold v (Wf negated on host)
                    s1 = wrk.tile([128, 2 * D], F32, name=f"s1_{rname}_{l}_{t}_{b}", tag="s1")
                    nc.scalar.activation(s1[:], psm[:], AF.Exp)
                    sp = wrk.tile([128, D], F32, name=f"sp_{rname}_{l}_{t}_{b}", tag="sp")
                    nc.scalar.activation(sp[:], s1[:, D:2 * D], AF.Ln, bias=1.0)
                    d1 = wrk.tile([128, D], F32, name=f"d1_{rname}_{l}_{t}_{b}", tag="d1")
                    nc.scalar.activation(d1[:], s1[:, 0:D], AF.Identity, bias=1.0)
                    rsg = wrk.tile([128, D], F32, name=f"rsg_{rname}_{l}_{t}_{b}", tag="rsg")
                    nc.vector.reciprocal_approx_fast(rsg[:], d1[:])
                    m = wrk.tile([128, D], BF16, name=f"m_{rname}_{l}_{t}_{b}", tag="m")
                    nc.vector.tensor_tensor(m[:], rsg[:], sp[:], op=OP.mult)
                    if len(pend) >= 2:
                        pend.pop(0)()
                    pend.append(
                        (lambda ohe=ohe, m=m, first=first, last=last:
                            nc.tensor.matmul(pagg[:], ohe, m[:], start=first, stop=last)))
                while pend:
                    pend.pop(0)()

                # -------- tile epilogue: ACC = cg_agg + residual (cg first)
                asl = ACC[dty][:, t * D:(t + 1) * D]
                nc.vector.scalar_tensor_tensor(
                    asl, pagg[:], 1.0, xres[dty][:, t * D:(t + 1) * D],
                    op0=OP.mult, op1=OP.add)

            def type_epilogue(ty, tyi, l, ACC, last_layer):
                accT = epi.tile([128, TILES * D], BF16, name=f"accT_{ty}_{l}", tag="accT")
                for t in range(TILES):
                    ptr = pzp.tile([128, 128], BF16, name=f"ptr_{ty}_{l}_{t}", tag="pz")
                    nc.tensor.transpose(ptr[:], ACC[ty][:, t * D:(t + 1) * D], ident_b[:])
                    nc.scalar.copy(accT[:, t * D:(t + 1) * D], ptr[:])
                for k in range(TILES * D // 512):
                    pnw = paggp.tile([128, 512], F32, name=f"pnw_{ty}_{l}_{k}", tag="pagg")
                    nc.tensor.matmul(pnw[:], nw_w[:, l * D:(l + 1) * D],
                                     accT[:, k * 512:(k + 1) * 512],
                                     start=True, stop=True)
                    if last_layer:
                        osb = epi.tile([128, 512], F32, name=f"osb_{ty}_{l}_{k}", tag="osb")
                        nc.scalar.activation(osb[:], pnw[:], AF.Identity,
                                             bias=nw_b[:, l:l + 1])
                        nc.sync.dma_start(dr[f"out_{ty}"][:, k * 512:(k + 1) * 512], osb[:])
                    else:
                        nc.scalar.activation(xfm[ty][:, k * 512:(k + 1) * 512], pnw[:],
                                             AF.Identity, bias=nw_b[:, l:l + 1])
                if not last_layer:
                    # back to dst-major for residuals + halo exchange
                    for t in range(TILES):
                        ptr2 = pzp.tile([128, 128], BF16, name=f"ptr2_{ty}_{l}_{t}", tag="pz")
                        nc.tensor.transpose(ptr2[:], xfm[ty][:, t * D:(t + 1) * D], ident_b[:])
                        nc.vector.tensor_copy(xres[ty][:, t * D:(t + 1) * D], ptr2[:])
                    ag_in = drm.tile([128, TILES * D], BF16, name=f"agin_{ty}_{l}", tag=f"agin_{ty}")
                    ag_out = drm.tile([CORES * 128, TILES * D], BF16,
                                      name=f"agout_{ty}_{l}", tag=f"agout_{ty}",
                                      addr_space="Shared")
                    nc.sync.dma_start(ag_in[:], xres[ty][:])
                    agins[ty] = (ag_in, ag_out)

            agins = {}
            for l in range(k_layers):
                last_layer = (l == k_layers - 1)
                ACC = {}
                for ty in ("my", "opp"):
                    ACC[ty] = accp.tile([128, TILES * D], BF16, name=f"acc_{ty}_{l}", tag=f"acc_{ty}")

                for (cgr, gatr, sty, dty) in (("loses", "beats", "my", "opp"),
                                              ("rev_beats", "rev_loses", "opp", "my")):
                    chunks = {}
                    for q in range(NCH):
                        chunks[(cgr, q)] = gather_chunk(cgr, sty, l, q)
                        chunks[(gatr, q)] = gather_chunk(gatr, sty, l, q)
                    for t in range(TILES):
                        q, tq = t // GTILES, t % GTILES
                        cg_tile(cgr, dty, l, t, tq, ACC, *chunks[(cgr, q)])
                        gat_tile(gatr, dty, l, t, tq, ACC, *chunks[(gatr, q)])

                for tyi, ty in enumerate(("my", "opp")):
                    if ty not in {r[3] for r in rels_active}:
                        continue
                    type_epilogue(ty, tyi, l, ACC, last_layer)
                if not last_layer:
                    for ty in ("my", "opp"):
                        ag_in, ag_out = agins.pop(ty)
                        nc.gpsimd.collective_compute(
                            "AllGather", mybir.AluOpType.bypass,
                            replica_groups=[list(range(CORES))],
                            ins=[ag_in.opt()], outs=[ag_out.opt()],
                        )
                        nc.sync.dma_start(
                            xw[ty][:].rearrange("p (c j) -> p c j", c=CORES),
                            ag_out[:].rearrange("(c p) j -> p c j", p=128),
                        )


    nc.compile()
    return nc


_prog_cache = {}


def _get_program(Bmax):
    if Bmax not in _prog_cache:
        _prog_cache[Bmax] = _build_program(Bmax)
    return _prog_cache[Bmax]


# ------------------------------------------------------------------- kernel

def kernel(**inputs):
    global LAST_EXEC_NS
    from concourse.bass_utils import run_bass_kernel_spmd

    f32 = lambda k: np.asarray(inputs[k], np.float32)
    x_my, x_opp = f32("x_my"), f32("x_opp")

    # edges
    eprep = {}
    Bmax = 1
    for rname, key in (("loses", "ei_loses"), ("beats", "ei_beats"),
                       ("rev_beats", "ei_rev_beats"), ("rev_loses", "ei_rev_loses")):
        percore, mc = _prep_edges(np.asarray(inputs[key]))
        eprep[rname] = percore
        Bmax = max(Bmax, -(-mc // 128))
    packed = {r: _pack_edges(eprep[r], Bmax) for r in eprep}

    nc = _get_program(Bmax)

    # shared (per-core identical) tensors
    shared = {}
    shared["xw_my"] = _wrap_nodes(x_my)
    shared["xw_opp"] = _wrap_nodes(x_opp)
    for rname, kind, _, _ in RELS:
        tag = {"loses": "cg_lose", "beats": "gat_beats",
               "rev_beats": "cg_rev", "rev_loses": "gat_rev"}[rname]
        if kind == "gat":
            shared[f"wl_{rname}"] = np.ascontiguousarray(f32(f"{tag}_Wl")).astype(BF)
            shared[f"wr_{rname}"] = np.ascontiguousarray(f32(f"{tag}_Wr")).astype(BF)
            att = f32(f"{tag}_att")  # [L, H, D]
            shared[f"att_{rname}"] = np.stack(
                [_rep(att[l].reshape(-1)) for l in range(L)]).astype(BF)
            b = f32(f"{tag}_b")  # [L, D]
            shared[f"gb_{rname}"] = np.stack([_rep(b[l]) for l in range(L)])
        else:
            # f-gate (Wf) negated so psm[:, :D] = -u and sigmoid(u) = 1/(1+e^{psm0})
            wf, ws = -f32(f"{tag}_Wf"), f32(f"{tag}_Ws")  # [L, 2D, D]
            shared[f"wt_{rname}"] = np.ascontiguousarray(
                np.concatenate([wf[:, :D, :], ws[:, :D, :]], axis=2)).astype(BF)
            shared[f"wb_{rname}"] = np.ascontiguousarray(
                np.concatenate([wf[:, D:, :], ws[:, D:, :]], axis=2)).astype(BF)
            bfv, bsv = -f32(f"{tag}_bf"), f32(f"{tag}_bs")  # [L, D]
            shared[f"cb_{rname}"] = np.ascontiguousarray(
                np.concatenate([bfv, bsv], axis=1).reshape(L, 1, 2 * D)).astype(BF)
    shared["nw_w"] = np.ascontiguousarray(f32("nw_W")).astype(BF)
    shared["nw_b"] = np.ascontiguousarray(f32("nw_b").reshape(L, 128, 1))
    shared["ident_f"] = np.eye(128, dtype=np.float32)
    shared["ident_b"] = np.eye(128).astype(BF)

    in_maps = []
    for c in range(CORES):
        m = dict(shared)
        m["xres_my"] = _dst_major_slice(x_my, c)
        m["xres_opp"] = _dst_major_slice(x_opp, c)
        m["xfm_my"] = _feat_major_slice(x_my, c)
        m["xfm_opp"] = _feat_major_slice(x_opp, c)
        for rname in packed:
            s_a, l_a = packed[rname][c]
            m[f"si_{rname}"] = _idx_dev(s_a)
            ohe, ohd = _onehots(l_a, Bmax)
            m[f"ohe_{rname}"] = ohe
            m[f"ohd_{rname}"] = ohd
        in_maps.append(m)

    trace = os.environ.get("KERNEL_PROFILE", "0") == "1"
    res = run_bass_kernel_spmd(nc, in_maps, core_ids=list(range(CORES)),
                               trace=trace, trace_cores=[0] if trace else None)
    LAST_EXEC_NS = res.exec_time_ns

    global DBG
    DBG = res.results

    def unshard(key):
        # per-core [128 f, TILES*128 node] f32 -> [N, D]
        parts = []
        for c in range(CORES):
            a = res.results[c][key]  # [128, 2560]
            parts.append(np.ascontiguousarray(
                a.reshape(D, TILES, 128).transpose(1, 2, 0).reshape(SHARD, D)))
        return np.concatenate(parts)[:N]

    return unshard("out_my"), unshard("out_opp")
